# revision 35
# baseline (speedup 1.0000x reference)
"""Multi-head self-attention (B=2, S=2048, D=1024, H=16) on 8 TRN2 NeuronCores.

Tensor-parallel over heads: each core owns 2 heads. Accepts FULL inputs,
returns FULL output. Host pre-transposes x and slices per-head weights;
each core computes qkv -> per-head LayerNorm -> attention -> partial
output projection (over its 128 embed dims); host sums the 8 partials.
"""

import os
import sys

import numpy as np

for _p in ("/opt/trn_rl_repo", "/root/.axon_site/_ro/trn_rl_repo"):
    if os.path.isdir(_p) and _p not in sys.path:
        sys.path.insert(0, _p)
        break

import concourse.bass as bass  # noqa: E402
import concourse.bacc as bacc  # noqa: E402
import concourse.tile as tile  # noqa: E402
from concourse import mybir  # noqa: E402
from concourse.bass_utils import run_bass_kernel_spmd  # noqa: E402

F32 = mybir.dt.float32
F32R = mybir.dt.float32r
BF16 = mybir.dt.bfloat16
AF = mybir.ActivationFunctionType

NCORES = 8
D = 1024
H = 16
HD = 64
HPC = H // NCORES          # heads per core = 2
DPC = HPC * HD             # embed dims per core = 128
EPS = 1e-5


def build_nc(B, S, affine):
    """Build the SPMD Bass program for one core (same program, 8 cores)."""
    T = B * S                      # total token columns
    NTB = T // 128                 # 128-token blocks
    NCH = T // 512                 # 512-token chunks
    QC = S // 512                  # q-chunks per batch
    KB = S // 128                  # k-blocks per batch
    KCH = D // 128                 # contraction chunks (8)
    SCALE = 1.0 / np.sqrt(HD)

    nc = bacc.Bacc(
        "TRN2",
        target_bir_lowering=False,
        debug=False,
        enable_asserts=True,
        num_devices=NCORES,
    )

    # xTb[n, p, c, u] = x^T[c*128+p, n*512+u]  (per-chunk blocked: each
    # chunk DMA is 128 partitions x 8KB contiguous)
    xTb = nc.dram_tensor("xTb", [NCH, 128, KCH, 512], BF16, kind="ExternalInput").ap()
    # wqb[p, c, j] = w_qkv_slice^T[c*128+p, j]
    wqb = nc.dram_tensor("wqb", [128, KCH, 3 * DPC], BF16, kind="ExternalInput").ap()
    bq = nc.dram_tensor("b_qkv_s", [1, 3 * DPC], BF16, kind="ExternalInput").ap()
    wp = nc.dram_tensor("wt_proj", [DPC, D], BF16, kind="ExternalInput").ap()
    bpb = nc.dram_tensor("c_bpb", [128, D], BF16, kind="ExternalInput").ap()
    onesb = nc.dram_tensor("c_onesb", [1, 512], BF16, kind="ExternalInput").ap()
    eye = nc.dram_tensor("c_eye", [128, 128], BF16, kind="ExternalInput").ap()
    if affine:
        gb = nc.dram_tensor("c_gb", [128, 4, HD], F32, kind="ExternalInput").ap()
    outp = nc.dram_tensor("outp", [T, D], BF16, kind="ExternalOutput").ap()

    from contextlib import ExitStack

    with tile.TileContext(nc) as tc, ExitStack() as stack:
        const = stack.enter_context(tc.tile_pool(name="const", bufs=1))
        persist = stack.enter_context(tc.tile_pool(name="persist", bufs=1))

        # phase-1-critical constants first; wp/bpb (phase 2 only) last so
        # their DMA issue doesn't delay the first qkv matmul
        wq_sb = const.tile([128, KCH, 3 * DPC], BF16, tag="wq")
        nc.sync.dma_start(out=wq_sb, in_=wqb)
        bq_sb = const.tile([1, 3 * DPC], BF16, tag="bq")
        nc.sync.dma_start(out=bq_sb, in_=bq)
        onesb_sb = const.tile([1, 512], BF16, tag="onesb")
        nc.sync.dma_start(out=onesb_sb, in_=onesb)
        eye_sb = const.tile([128, 128], BF16, tag="eye")
        nc.sync.dma_start(out=eye_sb, in_=eye)
        wp_sb = const.tile([DPC, D], BF16, tag="wp")
        nc.sync.dma_start(out=wp_sb, in_=wp)
        bpb_sb = const.tile([128, D], BF16, tag="bpb")
        nc.sync.dma_start(out=bpb_sb, in_=bpb)
        eps_sb = const.tile([128, 1], F32, tag="eps")
        nc.vector.memset(eps_sb, EPS)

        # dummy broadcast: forces the GpSimd 'attn' library load during the
        # prologue instead of on the first real denominator broadcast
        gwarm = const.tile([8, 1], F32, tag="gwarm")
        nc.gpsimd.partition_broadcast(gwarm, eps_sb[0:1, :], channels=8)

        if affine:
            gb_sb = const.tile([128, 4, HD], F32, tag="gb")
            nc.sync.dma_start(out=gb_sb, in_=gb)

        # persistent intermediates
        qT = persist.tile([128, T], BF16, tag="qT")     # [2h*64, tok] LN'd q^T
        kT = persist.tile([128, T], BF16, tag="kT")
        vO = persist.tile([128, HPC, NTB, HD + 1], BF16, tag="vO")  # v + ones col
        aT = persist.tile([128, T], BF16, tag="aT")     # attention out^T
        nc.vector.memset(vO[:, :, :, HD : HD + 1], 1.0)

        # ---------------- shared SBUF pools -----------------------------
        with (
            tc.tile_pool(name="xt", bufs=3) as xt_pool,
            tc.tile_pool(name="stage1", bufs=6) as stage1,
            tc.tile_pool(name="stats", bufs=6) as stats_pool,
            tc.tile_pool(name="exps", bufs=16) as exps,
            tc.tile_pool(name="stage2", bufs=3) as stage2,
            tc.tile_pool(name="ostage", bufs=8) as ostage,
        ):
            def phase1_start(n):
                xt = xt_pool.tile([128, KCH, 512], BF16, tag="xt")
                nc.sync.dma_start(out=xt, in_=xTb[n])
                return xt

            def phase1_block(n, tbl, xt, ps_alloc, tp_alloc):
                """qkv matmul + per-head LayerNorm + transpose for one
                128-token block. PSUM tiles come from the given allocators
                so this body can run standalone (own pools) or share the
                attention pool's scratch ring when interleaved."""
                if True:
                    tb = n * 4 + tbl
                    ps = ps_alloc()
                    nc.tensor.matmul(
                        ps,
                        lhsT=onesb_sb[0:1, 0:128],
                        rhs=bq_sb,
                        start=True,
                        stop=False,
                    )
                    for k in range(KCH):
                        nc.tensor.matmul(
                            ps,
                            lhsT=xt[:, k, tbl * 128 : (tbl + 1) * 128],
                            rhs=wq_sb[:, k, :],
                            start=False,
                            stop=(k == KCH - 1),
                        )
                    # LayerNorm over each head's 64 dims of q and k
                    qk = ps[:, 0 : 2 * DPC].rearrange("p (g d) -> p g d", d=HD)
                    st = stats_pool.tile([128, 4, 6], F32, tag="st")
                    mv = stats_pool.tile([128, 4, 2], F32, tag="mv")
                    for g in range(4):
                        nc.vector.bn_stats(out=st[:, g, :], in_=qk[:, g, :])
                        nc.vector.bn_aggr(out=mv[:, g, :], in_=st[:, g, :])
                    rstd = stats_pool.tile([128, 4], F32, tag="rstd")
                    nc.scalar.activation(
                        out=rstd, in_=mv[:, :, 1], func=AF.Sqrt, bias=eps_sb
                    )
                    nc.vector.reciprocal(out=rstd, in_=rstd)
                    qn = stage1.tile([128, 128], BF16, tag="qn")
                    kn = stage1.tile([128, 128], BF16, tag="kn")
                    for g in range(4):
                        dst = qn if g < 2 else kn
                        dsl = dst[:, (g % 2) * HD : (g % 2 + 1) * HD]
                        nc.vector.tensor_scalar(
                            out=dsl,
                            in0=qk[:, g, :],
                            scalar1=mv[:, g, 0:1],
                            scalar2=rstd[:, g : g + 1],
                            op0=mybir.AluOpType.subtract,
                            op1=mybir.AluOpType.mult,
                        )
                        if affine:
                            nc.vector.tensor_mul(dsl, dsl, gb_sb[:, 2 * (g // 2), :])
                            nc.vector.tensor_add(
                                dsl, dsl, gb_sb[:, 2 * (g // 2) + 1, :]
                            )
                    # v (+ ones col already set): copied on the ACT
                    # engine (COPY is in every table set) so the DVE does
                    # not lag the PE at the end of phase 1 and trip the
                    # HAM throttle into the phase-2 entry
                    for h in range(HPC):
                        nc.scalar.copy(
                            out=vO[:, h, tb, 0:HD],
                            in_=ps[:, 2 * DPC + h * HD : 2 * DPC + (h + 1) * HD],
                        )
                    # transpose q,k into [dim, token] layout
                    tp = tp_alloc()
                    nc.tensor.transpose(tp[:, 0:128], qn, eye_sb)
                    nc.tensor.transpose(tp[:, 128:256], kn, eye_sb)
                    ts = slice(tb * 128, (tb + 1) * 128)
                    nc.scalar.copy(out=qT[:, ts], in_=tp[:, 0:128])
                    nc.scalar.copy(out=kT[:, ts], in_=tp[:, 128:256])

            # ---- Region 1: batch 0's qkv/LN (nothing to overlap yet) ----
            with (
                tc.tile_pool(name="qkv_ps", bufs=4, space="PSUM") as qkv_ps,
                tc.tile_pool(name="t_ps", bufs=3, space="PSUM") as t_ps,
            ):
                for n in range(NCH):
                    xt1 = phase1_start(n)
                    for tbl in range(4):
                        phase1_block(
                            n, tbl, xt1,
                            lambda: qkv_ps.tile(
                                [128, 3 * DPC], F32, tag="ps", name="ps"
                            ),
                            lambda: t_ps.tile(
                                [128, 256], BF16, tag="tp", name="tp"
                            ),
                        )

            # ---- Regions 2+3: attention; batch 1's qkv/LN interleaved
            # under batch 0's ACT-bound attention chunks ----
            with (
                tc.tile_pool(name="sc_ps", bufs=2, space="PSUM") as sc_ps,
                tc.tile_pool(name="o_ps", bufs=1, space="PSUM") as o_ps,
            ):
                def proj_piece(tb, nn, ob):
                    pps = sc_ps.tile([128, 512], F32, tag="pp", name="pps")
                    nc.tensor.matmul(
                        pps,
                        lhsT=aT[:, tb * 128 : (tb + 1) * 128],
                        rhs=wp_sb[:, nn * 512 : (nn + 1) * 512],
                        start=True,
                        stop=True,
                    )
                    # bias folded into the eviction (bias was pre-
                    # broadcast across partitions on the host)
                    nc.vector.tensor_add(
                        ob[:, nn * 512 : (nn + 1) * 512],
                        pps,
                        bpb_sb[:, nn * 512 : (nn + 1) * 512],
                    )
                    if nn == D // 512 - 1:
                        nc.sync.dma_start(
                            out=outp[tb * 128 : (tb + 1) * 128, :], in_=ob
                        )

                def make_proj_thunks(b, qc):
                    # one thunk per projection matmul so the 8 MMs of a
                    # chunk's projection can be sprinkled one-per-kb into
                    # the steady state instead of landing as a clump that
                    # stalls the exp stream (and runs HAM-cold)
                    thunks = []
                    for tbl in range(4):
                        tb = (b * QC + qc) * 4 + tbl
                        holder = []

                        def t0(tb=tb, holder=holder):
                            ob = ostage.tile([128, D], BF16, tag="ob", name="ob")
                            holder.append(ob)
                            proj_piece(tb, 0, ob)

                        def t1(tb=tb, holder=holder):
                            proj_piece(tb, 1, holder[0])

                        thunks += [t0, t1]
                    return thunks

                def emit_proj(b, qc):
                    for t in make_proj_thunks(b, qc):
                        t()

                def pp_ps_alloc():
                    return sc_ps.tile([128, 3 * DPC], F32, tag="pp", name="ps")

                def pp_tp_alloc():
                    return sc_ps.tile([128, 256], BF16, tag="pp", name="tp")

                def emit_attnv(pend):
                    pex, poom, pgkb, pkb = pend
                    for h in range(HPC):
                        nc.tensor.matmul(
                            poom[:, h, :],
                            lhsT=vO[:, h, pgkb, :],
                            rhs=pex[:, h, :],
                            start=(pkb == 0),
                            stop=(pkb == KB - 1),
                        )

                def emit_epilogue_final(poom, pcols):
                    # drain variant: nothing follows, so normalize straight
                    # from oom; the den copy runs on the idle ACT so the
                    # DVE reciprocal starts as soon as it lands
                    den = stage2.tile([1, HPC, 512], F32, tag="den", name="den")
                    nc.scalar.copy(out=den, in_=poom[HD : HD + 1, :, :])
                    rc = stage2.tile([1, HPC, 512], F32, tag="rc", name="rc")
                    nc.vector.reciprocal_approx_fast(out=rc, in_=den)
                    rcb = stage2.tile([128, HPC, 512], F32, tag="rcb", name="rcb")
                    nc.gpsimd.partition_broadcast(rcb, rc, channels=128)
                    for h in range(HPC):
                        nc.vector.tensor_mul(
                            aT[h * HD : (h + 1) * HD, pcols],
                            poom[0:HD, h, :],
                            rcb[h * HD : (h + 1) * HD, h, :],
                        )

                def emit_epilogue(poom, pcols):
                    # release oom with ONE [65, 1024] copy to SBUF staging
                    # (the previous den-row + per-head copies were three
                    # serial DVE ops ~3.2us -- the den row runs on a single
                    # lane); everything downstream (reciprocal, broadcast,
                    # normalize into aT) reads the staging copy off the
                    # oom critical path. (custom DVE ops read PSUM
                    # incorrectly on HW, so the recip input must be SBUF
                    # anyway.)
                    stg = stage2.tile(
                        [HD + 1, HPC, 512], F32, tag="stg", name="stg"
                    )
                    nc.vector.tensor_copy(out=stg, in_=poom)
                    # custom DVE ops also mis-read at nonzero base
                    # partitions: bounce the den row to a base-0 tile
                    # (off the oom critical path -- it reads stg)
                    den = stage2.tile([1, HPC, 512], F32, tag="den", name="den")
                    nc.vector.tensor_copy(out=den, in_=stg[HD : HD + 1, :, :])
                    rc = stage2.tile([1, HPC, 512], F32, tag="rc", name="rc")
                    nc.vector.reciprocal_approx_fast(out=rc, in_=den)
                    rcb = stage2.tile([128, HPC, 512], F32, tag="rcb", name="rcb")
                    nc.gpsimd.partition_broadcast(rcb, rc, channels=128)
                    for h in range(HPC):
                        # rcb rows 0:64 (same broadcast value) keep both
                        # SBUF inputs at base partition 0
                        nc.vector.tensor_mul(
                            aT[h * HD : (h + 1) * HD, pcols],
                            stg[0:HD, h, :],
                            rcb[0:HD, h, :],
                        )

                # one-kb software pipeline ACROSS chunk boundaries: scores
                # for (ci, 0..1) are emitted before the previous chunk's
                # last attnV and epilogue, so the ACT's exp stream never
                # waits for the serial epilogue chain. The projection of
                # chunk ci-2 is placed exactly in the oom-drain shadow --
                # the PE would otherwise head-of-line stall there on
                # attnV(ci, 0) waiting for the single-buffered oom.
                chunks = [(b, qc) for b in range(B) for qc in range(QC)]
                pend = None       # (ex, oom, gkb, kb) awaiting its attnV
                pend_epi = None   # (oom, cols) awaiting its epilogue
                pthunks = []      # projection pieces of chunk ci-2
                late = []         # chunk[-2]'s pieces, trickled into the
                                  # last chunk's kb loop to shorten the
                                  # serial (HAM-cold) tail
                for ci, (b, qc) in enumerate(chunks):
                    cols = slice(b * S + qc * 512, b * S + (qc + 1) * 512)
                    oom = o_ps.tile([HD + 1, HPC, 512], F32, tag="o", name="oom")
                    if ci >= 2:
                        pthunks = make_proj_thunks(*chunks[ci - 2])
                    if ci == len(chunks) - 1 and len(chunks) >= 2:
                        late = make_proj_thunks(*chunks[ci - 1])
                    for kb in range(KB):
                        gkb = b * KB + kb
                        ks = slice(gkb * 128, (gkb + 1) * 128)
                        # two heads' score matmuls back-to-back: K=64 each
                        # at partition bases 0/64 -> disjoint row groups,
                        # so the PE runs them concurrently; both heads'
                        # scores in one PSUM tile for a single wide exp
                        scp = sc_ps.tile(
                            [128, HPC, 512], F32, tag="s", name="scp"
                        )
                        for h in range(HPC):
                            hp = slice(h * HD, (h + 1) * HD)
                            nc.tensor.matmul(
                                scp[:, h, :],
                                lhsT=kT[hp, ks],
                                rhs=qT[hp, cols],
                                start=True,
                                stop=True,
                            )
                        ex = exps.tile(
                            [128, HPC, 512], BF16, tag="ex", name="ex"
                        )
                        nc.scalar.activation(
                            out=ex, in_=scp, func=AF.Exp, scale=SCALE
                        )
                        if kb == 1 and pthunks:
                            for t in pthunks:
                                t()
                            pthunks = []
                        if pend is not None:
                            emit_attnv(pend)
                            if pend[3] == KB - 1:
                                emit_epilogue(*pend_epi)
                        if late and kb >= 8:
                            late.pop(0)()
                        pend = (ex, oom, gkb, kb)
                    pend_epi = (oom, cols)
                # drain: last attnV, its epilogue, final projections
                emit_attnv(pend)
                emit_epilogue_final(*pend_epi)
                for t in late:
                    t()
                emit_proj(*chunks[-1])

    nc.compile()
    return nc


def make_in_maps(x, w_qkv, b_qkv, w_proj, b_proj, q_gamma, q_beta, k_gamma, k_beta,
                 affine):
    import ml_dtypes
    bf = ml_dtypes.bfloat16

    B, S, _ = x.shape
    T = B * S
    NCH = T // 512
    KCH = D // 128
    xT = np.ascontiguousarray(x.reshape(T, D).T)
    # xTb[n, p, c, u] = xT[c*128+p, n*512+u]
    xTb = np.ascontiguousarray(
        xT.reshape(KCH, 128, NCH, 512).transpose(2, 1, 0, 3)
    ).astype(bf)
    eye = np.eye(128, dtype=np.float32)
    in_maps = []
    for c in range(NCORES):
        rs = slice(c * DPC, (c + 1) * DPC)
        w_slice = np.concatenate(
            [w_qkv[rs], w_qkv[D:2 * D][rs.start:rs.stop], w_qkv[2 * D:][rs.start:rs.stop]],
            axis=0,
        )  # [384, 1024]
        b_slice = np.concatenate(
            [b_qkv[rs], b_qkv[D:2 * D][rs.start:rs.stop], b_qkv[2 * D:][rs.start:rs.stop]]
        )[None, :]  # [1, 384]
        wT = np.ascontiguousarray(w_slice.T)  # [1024, 384]
        # wqb[p, c, j] = wT[c*128+p, j]
        wqb = np.ascontiguousarray(
            wT.reshape(KCH, 128, 3 * DPC).transpose(1, 0, 2)
        ).astype(bf)
        m = {
            "xTb": xTb,
            "wqb": wqb,
            "b_qkv_s": np.ascontiguousarray(b_slice).astype(bf),
            "wt_proj": np.ascontiguousarray(w_proj[:, rs].T).astype(bf),
            "c_bpb": np.ascontiguousarray(np.broadcast_to(
                b_proj[None, :] if c == 0 else np.zeros((1, D), np.float32),
                (128, D))).astype(bf),
            "c_onesb": np.ones((1, 512), bf),
            "c_eye": eye.astype(bf),
        }
        if affine:
            gb = np.stack([q_gamma, q_beta, k_gamma, k_beta])  # [4, 64]
            m["c_gb"] = np.ascontiguousarray(
                np.broadcast_to(gb[None], (128, 4, HD)).astype(np.float32)
            )
        in_maps.append(m)
    return in_maps


_NC_CACHE = {}

LAST_RESULTS = None


def kernel(x, w_qkv, b_qkv, w_proj, b_proj, q_gamma, q_beta, k_gamma, k_beta,
           **unused):
    global LAST_RESULTS
    x = np.asarray(x, np.float32)
    w_qkv = np.asarray(w_qkv, np.float32)
    b_qkv = np.asarray(b_qkv, np.float32)
    w_proj = np.asarray(w_proj, np.float32)
    b_proj = np.asarray(b_proj, np.float32)
    q_gamma = np.asarray(q_gamma, np.float32)
    q_beta = np.asarray(q_beta, np.float32)
    k_gamma = np.asarray(k_gamma, np.float32)
    k_beta = np.asarray(k_beta, np.float32)

    B, S, _ = x.shape
    affine = not (
        np.all(q_gamma == 1) and np.all(k_gamma == 1)
        and np.all(q_beta == 0) and np.all(k_beta == 0)
    )
    key = (B, S, affine)
    if key not in _NC_CACHE:
        _NC_CACHE[key] = build_nc(B, S, affine)
    nc = _NC_CACHE[key]

    in_maps = make_in_maps(
        x, w_qkv, b_qkv, w_proj, b_proj, q_gamma, q_beta, k_gamma, k_beta, affine
    )
    trace = bool(int(os.environ.get("BASS_KERNEL_TRACE", "0")))
    res = run_bass_kernel_spmd(
        nc, in_maps, core_ids=list(range(NCORES)), trace=trace
    )
    LAST_RESULTS = res
    acc = np.zeros((B * S, D), np.float32)
    for r in res.results:
        acc += np.asarray(r["outp"], dtype=np.float32)
    return acc.reshape(B, S, D)


# revision 36
# speedup vs baseline: 1.0051x; 1.0051x over previous
"""Multi-head self-attention (B=2, S=2048, D=1024, H=16) on 8 TRN2 NeuronCores.

Tensor-parallel over heads: each core owns 2 heads. Accepts FULL inputs,
returns FULL output. Host pre-transposes x and slices per-head weights;
each core computes qkv -> per-head LayerNorm -> attention -> partial
output projection (over its 128 embed dims); host sums the 8 partials.
"""

import os
import sys

import numpy as np

for _p in ("/opt/trn_rl_repo", "/root/.axon_site/_ro/trn_rl_repo"):
    if os.path.isdir(_p) and _p not in sys.path:
        sys.path.insert(0, _p)
        break

import concourse.bass as bass  # noqa: E402
import concourse.bacc as bacc  # noqa: E402
import concourse.tile as tile  # noqa: E402
from concourse import mybir  # noqa: E402
from concourse.bass_utils import run_bass_kernel_spmd  # noqa: E402

F32 = mybir.dt.float32
F32R = mybir.dt.float32r
BF16 = mybir.dt.bfloat16
AF = mybir.ActivationFunctionType

NCORES = 8
D = 1024
H = 16
HD = 64
HPC = H // NCORES          # heads per core = 2
DPC = HPC * HD             # embed dims per core = 128
EPS = 1e-5


def build_nc(B, S, affine):
    """Build the SPMD Bass program for one core (same program, 8 cores)."""
    T = B * S                      # total token columns
    NTB = T // 128                 # 128-token blocks
    NCH = T // 512                 # 512-token chunks
    QC = S // 512                  # q-chunks per batch
    KB = S // 128                  # k-blocks per batch
    KCH = D // 128                 # contraction chunks (8)
    SCALE = 1.0 / np.sqrt(HD)

    nc = bacc.Bacc(
        "TRN2",
        target_bir_lowering=False,
        debug=False,
        enable_asserts=True,
        num_devices=NCORES,
    )

    # xTb[n, p, c, u] = x^T[c*128+p, n*512+u]  (per-chunk blocked: each
    # chunk DMA is 128 partitions x 8KB contiguous)
    xTb = nc.dram_tensor("xTb", [NCH, 128, KCH, 512], BF16, kind="ExternalInput").ap()
    # wqb[p, c, j] = w_qkv_slice^T[c*128+p, j]
    wqb = nc.dram_tensor("wqb", [128, KCH, 3 * DPC], BF16, kind="ExternalInput").ap()
    bq = nc.dram_tensor("b_qkv_s", [1, 3 * DPC], BF16, kind="ExternalInput").ap()
    wp = nc.dram_tensor("wt_proj", [DPC, D], BF16, kind="ExternalInput").ap()
    bpb = nc.dram_tensor("c_bpb", [128, D], BF16, kind="ExternalInput").ap()
    onesb = nc.dram_tensor("c_onesb", [1, 512], BF16, kind="ExternalInput").ap()
    eye = nc.dram_tensor("c_eye", [128, 128], BF16, kind="ExternalInput").ap()
    if affine:
        gb = nc.dram_tensor("c_gb", [128, 4, HD], F32, kind="ExternalInput").ap()
    outp = nc.dram_tensor("outp", [T, D], BF16, kind="ExternalOutput").ap()

    from contextlib import ExitStack

    with tile.TileContext(nc) as tc, ExitStack() as stack:
        const = stack.enter_context(tc.tile_pool(name="const", bufs=1))
        persist = stack.enter_context(tc.tile_pool(name="persist", bufs=1))

        # phase-1-critical constants first; wp/bpb (phase 2 only) last so
        # their DMA issue doesn't delay the first qkv matmul
        wq_sb = const.tile([128, KCH, 3 * DPC], BF16, tag="wq")
        nc.sync.dma_start(out=wq_sb, in_=wqb)
        bq_sb = const.tile([1, 3 * DPC], BF16, tag="bq")
        nc.sync.dma_start(out=bq_sb, in_=bq)
        onesb_sb = const.tile([1, 512], BF16, tag="onesb")
        nc.sync.dma_start(out=onesb_sb, in_=onesb)
        eye_sb = const.tile([128, 128], BF16, tag="eye")
        nc.sync.dma_start(out=eye_sb, in_=eye)
        wp_sb = const.tile([DPC, D], BF16, tag="wp")
        nc.sync.dma_start(out=wp_sb, in_=wp)
        bpb_sb = const.tile([128, D], BF16, tag="bpb")
        nc.sync.dma_start(out=bpb_sb, in_=bpb)
        eps_sb = const.tile([128, 1], F32, tag="eps")
        nc.vector.memset(eps_sb, EPS)

        # dummy broadcast: forces the GpSimd 'attn' library load during the
        # prologue instead of on the first real denominator broadcast
        gwarm = const.tile([8, 1], F32, tag="gwarm")
        nc.gpsimd.partition_broadcast(gwarm, eps_sb[0:1, :], channels=8)

        if affine:
            gb_sb = const.tile([128, 4, HD], F32, tag="gb")
            nc.sync.dma_start(out=gb_sb, in_=gb)

        # persistent intermediates
        qT = persist.tile([128, T], BF16, tag="qT")     # [2h*64, tok] LN'd q^T
        kT = persist.tile([128, T], BF16, tag="kT")
        vO = persist.tile([128, HPC, NTB, HD + 1], BF16, tag="vO")  # v + ones col
        aT = persist.tile([128, T], BF16, tag="aT")     # attention out^T
        nc.vector.memset(vO[:, :, :, HD : HD + 1], 1.0)

        # ---------------- shared SBUF pools -----------------------------
        with (
            tc.tile_pool(name="xt", bufs=3) as xt_pool,
            tc.tile_pool(name="stage1", bufs=6) as stage1,
            tc.tile_pool(name="stats", bufs=6) as stats_pool,
            tc.tile_pool(name="exps", bufs=16) as exps,
            tc.tile_pool(name="stage2", bufs=3) as stage2,
            tc.tile_pool(name="ostage", bufs=8) as ostage,
        ):
            def phase1_start(n):
                xt = xt_pool.tile([128, KCH, 512], BF16, tag="xt")
                nc.sync.dma_start(out=xt, in_=xTb[n])
                return xt

            def phase1_block(n, tbl, xt, ps_alloc, tp_alloc):
                """qkv matmul + per-head LayerNorm + transpose for one
                128-token block. PSUM tiles come from the given allocators
                so this body can run standalone (own pools) or share the
                attention pool's scratch ring when interleaved."""
                if True:
                    tb = n * 4 + tbl
                    ps = ps_alloc()
                    nc.tensor.matmul(
                        ps,
                        lhsT=onesb_sb[0:1, 0:128],
                        rhs=bq_sb,
                        start=True,
                        stop=False,
                    )
                    for k in range(KCH):
                        nc.tensor.matmul(
                            ps,
                            lhsT=xt[:, k, tbl * 128 : (tbl + 1) * 128],
                            rhs=wq_sb[:, k, :],
                            start=False,
                            stop=(k == KCH - 1),
                        )
                    # LayerNorm over each head's 64 dims of q and k
                    qk = ps[:, 0 : 2 * DPC].rearrange("p (g d) -> p g d", d=HD)
                    st = stats_pool.tile([128, 4, 6], F32, tag="st")
                    mv = stats_pool.tile([128, 4, 2], F32, tag="mv")
                    for g in range(4):
                        nc.vector.bn_stats(out=st[:, g, :], in_=qk[:, g, :])
                        nc.vector.bn_aggr(out=mv[:, g, :], in_=st[:, g, :])
                    rstd = stats_pool.tile([128, 4], F32, tag="rstd")
                    nc.scalar.activation(
                        out=rstd, in_=mv[:, :, 1], func=AF.Sqrt, bias=eps_sb
                    )
                    nc.vector.reciprocal(out=rstd, in_=rstd)
                    qn = stage1.tile([128, 128], BF16, tag="qn")
                    kn = stage1.tile([128, 128], BF16, tag="kn")
                    for g in range(4):
                        dst = qn if g < 2 else kn
                        dsl = dst[:, (g % 2) * HD : (g % 2 + 1) * HD]
                        nc.vector.tensor_scalar(
                            out=dsl,
                            in0=qk[:, g, :],
                            scalar1=mv[:, g, 0:1],
                            scalar2=rstd[:, g : g + 1],
                            op0=mybir.AluOpType.subtract,
                            op1=mybir.AluOpType.mult,
                        )
                        if affine:
                            nc.vector.tensor_mul(dsl, dsl, gb_sb[:, 2 * (g // 2), :])
                            nc.vector.tensor_add(
                                dsl, dsl, gb_sb[:, 2 * (g // 2) + 1, :]
                            )
                    # v (+ ones col already set): copied on the ACT
                    # engine (COPY is in every table set) so the DVE does
                    # not lag the PE at the end of phase 1 and trip the
                    # HAM throttle into the phase-2 entry
                    for h in range(HPC):
                        nc.scalar.copy(
                            out=vO[:, h, tb, 0:HD],
                            in_=ps[:, 2 * DPC + h * HD : 2 * DPC + (h + 1) * HD],
                        )
                    # transpose q,k into [dim, token] layout
                    tp = tp_alloc()
                    nc.tensor.transpose(tp[:, 0:128], qn, eye_sb)
                    nc.tensor.transpose(tp[:, 128:256], kn, eye_sb)
                    ts = slice(tb * 128, (tb + 1) * 128)
                    nc.scalar.copy(out=qT[:, ts], in_=tp[:, 0:128])
                    nc.scalar.copy(out=kT[:, ts], in_=tp[:, 128:256])

            # ---- Region 1: batch 0's qkv/LN (nothing to overlap yet) ----
            with (
                tc.tile_pool(name="qkv_ps", bufs=4, space="PSUM") as qkv_ps,
                tc.tile_pool(name="t_ps", bufs=3, space="PSUM") as t_ps,
            ):
                for n in range(NCH):
                    xt1 = phase1_start(n)
                    for tbl in range(4):
                        phase1_block(
                            n, tbl, xt1,
                            lambda: qkv_ps.tile(
                                [128, 3 * DPC], F32, tag="ps", name="ps"
                            ),
                            lambda: t_ps.tile(
                                [128, 256], BF16, tag="tp", name="tp"
                            ),
                        )

            # ---- Regions 2+3: attention; batch 1's qkv/LN interleaved
            # under batch 0's ACT-bound attention chunks ----
            with (
                tc.tile_pool(name="sc_ps", bufs=2, space="PSUM") as sc_ps,
                tc.tile_pool(name="o_ps", bufs=1, space="PSUM") as o_ps,
            ):
                def proj_piece(tb, nn, ob):
                    pps = sc_ps.tile([128, 512], F32, tag="pp", name="pps")
                    nc.tensor.matmul(
                        pps,
                        lhsT=aT[:, tb * 128 : (tb + 1) * 128],
                        rhs=wp_sb[:, nn * 512 : (nn + 1) * 512],
                        start=True,
                        stop=True,
                    )
                    # bias folded into the eviction (bias was pre-
                    # broadcast across partitions on the host)
                    nc.vector.tensor_add(
                        ob[:, nn * 512 : (nn + 1) * 512],
                        pps,
                        bpb_sb[:, nn * 512 : (nn + 1) * 512],
                    )
                    if nn == D // 512 - 1:
                        nc.sync.dma_start(
                            out=outp[tb * 128 : (tb + 1) * 128, :], in_=ob
                        )

                def make_proj_thunks(b, qc):
                    # one thunk per projection matmul so the 8 MMs of a
                    # chunk's projection can be sprinkled one-per-kb into
                    # the steady state instead of landing as a clump that
                    # stalls the exp stream (and runs HAM-cold)
                    thunks = []
                    for tbl in range(4):
                        tb = (b * QC + qc) * 4 + tbl
                        holder = []

                        def t0(tb=tb, holder=holder):
                            ob = ostage.tile([128, D], BF16, tag="ob", name="ob")
                            holder.append(ob)
                            proj_piece(tb, 0, ob)

                        def t1(tb=tb, holder=holder):
                            proj_piece(tb, 1, holder[0])

                        thunks += [t0, t1]
                    return thunks

                def emit_proj(b, qc):
                    for t in make_proj_thunks(b, qc):
                        t()

                def pp_ps_alloc():
                    return sc_ps.tile([128, 3 * DPC], F32, tag="pp", name="ps")

                def pp_tp_alloc():
                    return sc_ps.tile([128, 256], BF16, tag="pp", name="tp")

                def emit_attnv(pend):
                    pex, poom, pgkb, pkb = pend
                    for h in range(HPC):
                        nc.tensor.matmul(
                            poom[:, h, :],
                            lhsT=vO[:, h, pgkb, :],
                            rhs=pex[:, h, :],
                            start=(pkb == 0),
                            stop=(pkb == KB - 1),
                        )

                def emit_epilogue_final(poom, pcols):
                    # drain variant: nothing follows, so normalize straight
                    # from oom (no release urgency) -- shortens the serial
                    # tail chain by the aT-copy + in-place-mul passes
                    den = stage2.tile([1, HPC, 512], F32, tag="den", name="den")
                    nc.vector.tensor_copy(out=den, in_=poom[HD : HD + 1, :, :])
                    rc = stage2.tile([1, HPC, 512], F32, tag="rc", name="rc")
                    nc.vector.reciprocal_approx_fast(out=rc, in_=den)
                    rcb = stage2.tile([128, HPC, 512], F32, tag="rcb", name="rcb")
                    nc.gpsimd.partition_broadcast(rcb, rc, channels=128)
                    for h in range(HPC):
                        nc.vector.tensor_mul(
                            aT[h * HD : (h + 1) * HD, pcols],
                            poom[0:HD, h, :],
                            rcb[h * HD : (h + 1) * HD, h, :],
                        )

                def emit_epilogue(poom, pcols):
                    # release oom with ONE [65, 1024] copy to SBUF staging
                    # (the previous den-row + per-head copies were three
                    # serial DVE ops ~3.2us -- the den row runs on a single
                    # lane); everything downstream (reciprocal, broadcast,
                    # normalize into aT) reads the staging copy off the
                    # oom critical path. (custom DVE ops read PSUM
                    # incorrectly on HW, so the recip input must be SBUF
                    # anyway.)
                    stg = stage2.tile(
                        [HD + 1, HPC, 512], F32, tag="stg", name="stg"
                    )
                    nc.vector.tensor_copy(out=stg, in_=poom)
                    # custom DVE ops also mis-read at nonzero base
                    # partitions: bounce the den row to a base-0 tile
                    # (off the oom critical path -- it reads stg)
                    den = stage2.tile([1, HPC, 512], F32, tag="den", name="den")
                    nc.vector.tensor_copy(out=den, in_=stg[HD : HD + 1, :, :])
                    rc = stage2.tile([1, HPC, 512], F32, tag="rc", name="rc")
                    nc.vector.reciprocal_approx_fast(out=rc, in_=den)
                    rcb = stage2.tile([128, HPC, 512], F32, tag="rcb", name="rcb")
                    nc.gpsimd.partition_broadcast(rcb, rc, channels=128)
                    for h in range(HPC):
                        # rcb rows 0:64 (same broadcast value) keep both
                        # SBUF inputs at base partition 0
                        nc.vector.tensor_mul(
                            aT[h * HD : (h + 1) * HD, pcols],
                            stg[0:HD, h, :],
                            rcb[0:HD, h, :],
                        )

                # one-kb software pipeline ACROSS chunk boundaries: scores
                # for (ci, 0..1) are emitted before the previous chunk's
                # last attnV and epilogue, so the ACT's exp stream never
                # waits for the serial epilogue chain. The projection of
                # chunk ci-2 is placed exactly in the oom-drain shadow --
                # the PE would otherwise head-of-line stall there on
                # attnV(ci, 0) waiting for the single-buffered oom.
                chunks = [(b, qc) for b in range(B) for qc in range(QC)]
                pend = None       # (ex, oom, gkb, kb) awaiting its attnV
                pend_epi = None   # (oom, cols) awaiting its epilogue
                pthunks = []      # projection pieces of chunk ci-2
                late = []         # chunk[-2]'s pieces, trickled into the
                                  # last chunk's kb loop to shorten the
                                  # serial (HAM-cold) tail
                for ci, (b, qc) in enumerate(chunks):
                    cols = slice(b * S + qc * 512, b * S + (qc + 1) * 512)
                    oom = o_ps.tile([HD + 1, HPC, 512], F32, tag="o", name="oom")
                    if ci >= 2:
                        pthunks = make_proj_thunks(*chunks[ci - 2])
                    if ci == len(chunks) - 1 and len(chunks) >= 2:
                        late = make_proj_thunks(*chunks[ci - 1])
                    for kb in range(KB):
                        gkb = b * KB + kb
                        ks = slice(gkb * 128, (gkb + 1) * 128)
                        # two heads' score matmuls back-to-back: K=64 each
                        # at partition bases 0/64 -> disjoint row groups,
                        # so the PE runs them concurrently; both heads'
                        # scores in one PSUM tile for a single wide exp
                        scp = sc_ps.tile(
                            [128, HPC, 512], F32, tag="s", name="scp"
                        )
                        for h in range(HPC):
                            hp = slice(h * HD, (h + 1) * HD)
                            nc.tensor.matmul(
                                scp[:, h, :],
                                lhsT=kT[hp, ks],
                                rhs=qT[hp, cols],
                                start=True,
                                stop=True,
                            )
                        ex = exps.tile(
                            [128, HPC, 512], BF16, tag="ex", name="ex"
                        )
                        nc.scalar.activation(
                            out=ex, in_=scp, func=AF.Exp, scale=SCALE
                        )
                        if kb == 1 and pthunks:
                            for t in pthunks:
                                t()
                            pthunks = []
                        if pend is not None:
                            emit_attnv(pend)
                            if pend[3] == KB - 1:
                                emit_epilogue(*pend_epi)
                        if late and kb >= 8:
                            late.pop(0)()
                        pend = (ex, oom, gkb, kb)
                    pend_epi = (oom, cols)
                # drain: last attnV, its epilogue, final projections
                emit_attnv(pend)
                emit_epilogue_final(*pend_epi)
                for t in late:
                    t()
                emit_proj(*chunks[-1])

    nc.compile()
    return nc


def make_in_maps(x, w_qkv, b_qkv, w_proj, b_proj, q_gamma, q_beta, k_gamma, k_beta,
                 affine):
    import ml_dtypes
    bf = ml_dtypes.bfloat16

    B, S, _ = x.shape
    T = B * S
    NCH = T // 512
    KCH = D // 128
    xT = np.ascontiguousarray(x.reshape(T, D).T)
    # xTb[n, p, c, u] = xT[c*128+p, n*512+u]
    xTb = np.ascontiguousarray(
        xT.reshape(KCH, 128, NCH, 512).transpose(2, 1, 0, 3)
    ).astype(bf)
    eye = np.eye(128, dtype=np.float32)
    in_maps = []
    for c in range(NCORES):
        rs = slice(c * DPC, (c + 1) * DPC)
        w_slice = np.concatenate(
            [w_qkv[rs], w_qkv[D:2 * D][rs.start:rs.stop], w_qkv[2 * D:][rs.start:rs.stop]],
            axis=0,
        )  # [384, 1024]
        b_slice = np.concatenate(
            [b_qkv[rs], b_qkv[D:2 * D][rs.start:rs.stop], b_qkv[2 * D:][rs.start:rs.stop]]
        )[None, :]  # [1, 384]
        wT = np.ascontiguousarray(w_slice.T)  # [1024, 384]
        # wqb[p, c, j] = wT[c*128+p, j]
        wqb = np.ascontiguousarray(
            wT.reshape(KCH, 128, 3 * DPC).transpose(1, 0, 2)
        ).astype(bf)
        m = {
            "xTb": xTb,
            "wqb": wqb,
            "b_qkv_s": np.ascontiguousarray(b_slice).astype(bf),
            "wt_proj": np.ascontiguousarray(w_proj[:, rs].T).astype(bf),
            "c_bpb": np.ascontiguousarray(np.broadcast_to(
                b_proj[None, :] if c == 0 else np.zeros((1, D), np.float32),
                (128, D))).astype(bf),
            "c_onesb": np.ones((1, 512), bf),
            "c_eye": eye.astype(bf),
        }
        if affine:
            gb = np.stack([q_gamma, q_beta, k_gamma, k_beta])  # [4, 64]
            m["c_gb"] = np.ascontiguousarray(
                np.broadcast_to(gb[None], (128, 4, HD)).astype(np.float32)
            )
        in_maps.append(m)
    return in_maps


_NC_CACHE = {}

LAST_RESULTS = None


def kernel(x, w_qkv, b_qkv, w_proj, b_proj, q_gamma, q_beta, k_gamma, k_beta,
           **unused):
    global LAST_RESULTS
    x = np.asarray(x, np.float32)
    w_qkv = np.asarray(w_qkv, np.float32)
    b_qkv = np.asarray(b_qkv, np.float32)
    w_proj = np.asarray(w_proj, np.float32)
    b_proj = np.asarray(b_proj, np.float32)
    q_gamma = np.asarray(q_gamma, np.float32)
    q_beta = np.asarray(q_beta, np.float32)
    k_gamma = np.asarray(k_gamma, np.float32)
    k_beta = np.asarray(k_beta, np.float32)

    B, S, _ = x.shape
    affine = not (
        np.all(q_gamma == 1) and np.all(k_gamma == 1)
        and np.all(q_beta == 0) and np.all(k_beta == 0)
    )
    key = (B, S, affine)
    if key not in _NC_CACHE:
        _NC_CACHE[key] = build_nc(B, S, affine)
    nc = _NC_CACHE[key]

    in_maps = make_in_maps(
        x, w_qkv, b_qkv, w_proj, b_proj, q_gamma, q_beta, k_gamma, k_beta, affine
    )
    trace = bool(int(os.environ.get("BASS_KERNEL_TRACE", "0")))
    res = run_bass_kernel_spmd(
        nc, in_maps, core_ids=list(range(NCORES)), trace=trace
    )
    LAST_RESULTS = res
    acc = np.zeros((B * S, D), np.float32)
    for r in res.results:
        acc += np.asarray(r["outp"], dtype=np.float32)
    return acc.reshape(B, S, D)
